# revision 13
# baseline (speedup 1.0000x reference)
"""DGCNN Bass/Tile kernel for Trainium2 — 8-core data-parallel (1 point cloud per core).

Per edge-conv block (exact algebra):
  edge feat [ctr, nbr] @ W = ctr @ Wc + nbr @ Wn
  out[n] = max_k relu(bn(A[n] + B[idx[n,k]])) = relu((A[n] + max_k B[idx[n,k]]) * s + t)
  (s = g*rsqrt(v+eps) > 0, t = b - m*s; relu/max/affine commute since s > 0)

k-NN scores (monotone-equivalent to the reference's pd, per row):
  score[n, m] = 2<x_n, x_m> - |x_m|^2    (row-constant -|x_n|^2 dropped)
computed on PE (fp32) into PSUM, evicted by ACT, top-16 via
max8/max_index/match_replace on DVE (fp32-exact selection).

Gather of B rows from DRAM via gpsimd dma_gather (int16 idx wrapped into 16
partitions, replicated to the 8 Q7 cores), max over the 16 neighbor slots on
DVE, PE-transpose back to [D, N], fused BN+relu on ACT.
"""
import numpy as np
from contextlib import ExitStack

import concourse.bass as bass
import concourse.mybir as mybir
import concourse.tile as tile
from concourse import bacc
from concourse import bass_utils
from concourse.masks import make_identity

N = 2048
K = 16
EPS = 1e-3
NT = N // 128  # 16 row-tiles
BLOCKS = [(3, 64), (64, 64), (64, 128), (128, 256)]  # (C_in, D_out)

F32 = mybir.dt.float32
U16 = mybir.dt.uint16
I16 = mybir.dt.int16
AF = mybir.ActivationFunctionType


def build(nc: bass.Bass):
    # ---- DRAM I/O ----
    xT_d = nc.dram_tensor("xT", [3, N], F32, kind="ExternalInput")
    wc_d, wn_d, s_d, t_d = [], [], [], []
    for i, (C, D) in enumerate(BLOCKS):
        wc_d.append(nc.dram_tensor(f"Wc{i+1}", [C, D], F32, kind="ExternalInput"))
        wn_d.append(nc.dram_tensor(f"Wn{i+1}", [C, D], F32, kind="ExternalInput"))
        s_d.append(nc.dram_tensor(f"s{i+1}", [D, 1], F32, kind="ExternalInput"))
        t_d.append(nc.dram_tensor(f"t{i+1}", [D, 1], F32, kind="ExternalInput"))
    w5_d = nc.dram_tensor("W5s", [512, 512], F32, kind="ExternalInput")
    t5_d = nc.dram_tensor("t5", [1, 512], F32, kind="ExternalInput")
    out_d = nc.dram_tensor("out", [N, 512], F32, kind="ExternalOutput")

    with tile.TileContext(nc) as tc, ExitStack() as ctx:
        sb = ctx.enter_context(tc.tile_pool(name="sb", bufs=2))
        sb1 = ctx.enter_context(tc.tile_pool(name="sb1", bufs=1))
        ps = ctx.enter_context(tc.tile_pool(name="ps", bufs=2, space="PSUM"))
        psd = ctx.enter_context(tc.tile_pool(name="psd", bufs=1, space="PSUM"))
        dram = ctx.enter_context(tc.tile_pool(name="dram", bufs=1, space="DRAM"))

        ident = sb1.tile([128, 128], F32, tag="ident")
        make_identity(nc, ident[:])
        ones_row = sb1.tile([1, N], F32, tag="ones_row")
        nc.gpsimd.memset(ones_row[:], 1.0)
        ones_col = sb1.tile([128, 1], F32, tag="ones_col")
        nc.gpsimd.memset(ones_col[:], 1.0)

        # persistent xT holders for the final concat matmul
        x12T = sb1.tile([128, N], F32, tag="x12T")  # x1 rows 0:64, x2 rows 64:128
        x4aT = sb1.tile([128, N], F32, tag="x4aT")
        x4bT = sb1.tile([128, N], F32, tag="x4bT")

        # per-block Cc holders. Blocks 1-3: row 0 = -sq, rows 1:C+1 = x^T
        # (aug row leads so engine start-partitions stay 0-aligned).
        cc1 = sb1.tile([4, N], F32, tag="cc1")
        cc2 = sb1.tile([65, N], F32, tag="cc2")
        cc3 = sb1.tile([65, N], F32, tag="cc3")
        cc4 = sb1.tile([128, N], F32, tag="cc4")  # block4 x^T (= x3); reused as W5 K-chunk
        sq4 = sb1.tile([1, N], F32, tag="sq4")    # block4 -sq row

        nc.sync.dma_start(cc1[0:3, :], xT_d.ap())

        # (tile holding x^T at rows 0:C, aug -sq row index)
        xT_of = {1: cc1, 2: cc2, 3: cc3, 4: cc4}

        for i, (C, D) in enumerate(BLOCKS):
            blk = i + 1
            cct = xT_of[blk]
            xT = cct[0:C, :]

            # ---- prep: -|x_m|^2 row (row C of cct; block4: separate sq4) ----
            # Engine APs need 32-aligned base partitions; block1's row 3 (and
            # its ones row) are written via a partition-0 staging tile + DMA.
            xsq = sb1.tile([C, N], F32, tag="xsq")
            nc.scalar.activation(xsq[:], xT, AF.Square)
            sqstage = sb.tile([1, N], F32, tag="sqstage")
            for j in range(4):
                sqp = ps.tile([1, 512], F32, tag="pscratch")
                nc.tensor.matmul(sqp[:], ones_col[0:C, :],
                                 xsq[:, j * 512:(j + 1) * 512], start=True, stop=True)
                nc.scalar.activation(sqstage[:, j * 512:(j + 1) * 512],
                                     sqp[:], AF.Copy, scale=-1.0)
            sq_dst = sq4[0:1, :] if blk == 4 else cct[C:C + 1, :]
            nc.sync.dma_start(sq_dst, sqstage[:])

            # ---- prep: Rr = [2x^T; ones] (block4: ones via separate chunk) ----
            if blk < 4:
                rr = sb1.tile([C + 1, N], F32, tag="rr")
                nc.scalar.mul(rr[0:C, :], xT, 2.0)
                nc.sync.dma_start(rr[C:C + 1, :], ones_row[:])
                rr_chunks = [rr[0:C + 1, :]]
                cc_chunks = [cct[0:C + 1, :]]
            else:
                rr = sb1.tile([128, N], F32, tag="rr")
                nc.scalar.mul(rr[:], xT, 2.0)
                rr_chunks = [rr[:], ones_row[:]]
                cc_chunks = [cct[0:128, :], sq4[:]]

            # ---- prep: weights / bn params ----
            wc = sb.tile([C, D], F32, tag="wc")
            wn = sb.tile([C, D], F32, tag="wn")
            nc.sync.dma_start(wc[:], wc_d[i].ap())
            nc.sync.dma_start(wn[:], wn_d[i].ap())
            nch = (D + 127) // 128
            s_sb = sb.tile([128, nch], F32, tag="s_sb")
            t_sb = sb.tile([128, nch], F32, tag="t_sb")
            for c in range(nch):
                dw_ = min(128, D - c * 128)
                nc.sync.dma_start(s_sb[0:dw_, c:c + 1], s_d[i].ap()[c * 128:c * 128 + dw_, :])
                nc.sync.dma_start(t_sb[0:dw_, c:c + 1], t_d[i].ap()[c * 128:c * 128 + dw_, :])

            # ---- prep: A^T = Wc^T @ x^T [D, N] (raw; BN fused in epilogue) ----
            a_sb = sb1.tile([128, nch, N], F32, tag="a_sb")
            for dc in range(0, D, 128):
                dw = min(128, D - dc)
                for j in range(4):
                    ap_ = ps.tile([128, 512], F32, tag="pscratch")
                    nc.tensor.matmul(ap_[0:dw, :], wc[:, dc:dc + dw],
                                     xT[:, j * 512:(j + 1) * 512],
                                     start=True, stop=True)
                    nc.scalar.activation(
                        a_sb[0:dw, dc // 128, j * 512:(j + 1) * 512],
                        ap_[0:dw, :], AF.Copy)

            # ---- prep: B = x @ Wn row-major -> DRAM ----
            b_dram = dram.tile([N, D], F32, tag=f"b_dram{blk}")
            for t in range(NT):
                bp = ps.tile([128, D], F32, tag="pscratch")
                nc.tensor.matmul(bp[:], xT[:, t * 128:(t + 1) * 128], wn[:],
                                 start=True, stop=True)
                b_sb = sb.tile([128, D], F32, tag="b_sb")
                nc.scalar.activation(b_sb[:], bp[:], AF.Copy)
                nc.sync.dma_start(b_dram[t * 128:(t + 1) * 128, :], b_sb[:])

            # ---- main loop: pd -> top-16 -> gather -> slot-max ----
            Mall = sb1.tile([128, NT, D], F32, tag="Mall")
            nk = len(rr_chunks)
            for t in range(NT):
                pd = psd.tile([128, N], F32, tag="pd")
                for kc, (rc, cc) in enumerate(zip(rr_chunks, cc_chunks)):
                    for j in range(4):
                        nc.tensor.matmul(
                            pd[:, j * 512:(j + 1) * 512],
                            rc[:, t * 128:(t + 1) * 128],
                            cc[:, j * 512:(j + 1) * 512],
                            start=(kc == 0), stop=(kc == nk - 1))
                pdsb = sb.tile([128, N], F32, tag="pdsb")
                nc.scalar.activation(pdsb[:], pd[:], AF.Copy)

                v1 = sb.tile([128, 8], F32, tag="v1")
                v2 = sb.tile([128, 8], F32, tag="v2")
                itile = sb.tile([128, 16], U16, tag="itile")
                nc.vector.max(out=v1[:], in_=pdsb[:])
                nc.vector.max_index(out=itile[:, 0:8], in_max=v1[:], in_values=pdsb[:])
                nc.vector.match_replace(out=pdsb[:], in_to_replace=v1[:],
                                        in_values=pdsb[:], imm_value=-3e38)
                nc.vector.max(out=v2[:], in_=pdsb[:])
                nc.vector.max_index(out=itile[:, 8:16], in_max=v2[:], in_values=pdsb[:])

                # wrap: iw[q, m*8+g] = itile[g*16+q, m], replicated to all 8 cores
                iw = sb.tile([128, 128], I16, tag="iw")
                it16 = itile[:].bitcast(I16)
                for g in range(8):
                    nc.sync.dma_start(iw[0:16, g:g + 1 + 8 * 15:8],
                                      it16[g * 16:(g + 1) * 16, :])
                for r in range(1, 8):
                    nc.scalar.dma_start(iw[16 * r:16 * (r + 1), :], iw[0:16, :])

                gt = sb.tile([128, 16, D], F32, tag="gt")
                nc.gpsimd.dma_gather(
                    out_ap=gt[:], in_ap=b_dram[:], idxs_ap=iw[:],
                    num_idxs=N, num_idxs_reg=N, elem_size=D,
                    single_packet=False)
                nc.vector.tensor_reduce(
                    out=Mall[:, t:t + 1, :], in_=gt[:].rearrange("p k d -> p d k"),
                    op=mybir.AluOpType.max, axis=mybir.AxisListType.X)

            # ---- epilogue: x_next^T = relu((A^T + M^T) * s + t) ----
            if blk == 1:
                dsts, nxt = [(x12T, 0)], cc2
            elif blk == 2:
                dsts, nxt = [(x12T, 64)], cc3
            elif blk == 3:
                dsts, nxt = [(cc4, 0)], None
            else:
                dsts, nxt = [(x4aT, 0), (x4bT, 0)], None

            for dc in range(0, D, 128):
                dw = min(128, D - dc)
                dst, dst_off = dsts[dc // 128]
                for t in range(NT):
                    mtp = ps.tile([128, 128], F32, tag="pscratch")
                    nc.tensor.transpose(mtp[0:dw, :],
                                        Mall[:, t:t + 1, dc:dc + dw], ident[:])
                    tmp = sb.tile([128, 128], F32, tag="tmp_add")
                    nc.vector.tensor_add(
                        tmp[0:dw, :], mtp[0:dw, :],
                        a_sb[0:dw, dc // 128, t * 128:(t + 1) * 128])
                    nc.scalar.activation(
                        dst[dst_off:dst_off + dw, t * 128:(t + 1) * 128],
                        tmp[0:dw, :], AF.Relu,
                        scale=s_sb[0:dw, dc // 128:dc // 128 + 1],
                        bias=t_sb[0:dw, dc // 128:dc // 128 + 1])
                    if nxt is not None:
                        nc.vector.tensor_copy(
                            nxt[dc:dc + dw, t * 128:(t + 1) * 128],
                            dst[dst_off:dst_off + dw, t * 128:(t + 1) * 128])

        # ---- final: out = relu(x5aug @ W5aug) ----
        w5 = sb1.tile([128, 4, 512], F32, tag="w5")
        nc.sync.dma_start(w5[:], w5_d.ap().rearrange("(a c) d -> c a d", c=128))
        t5 = sb1.tile([1, 512], F32, tag="t5")
        nc.sync.dma_start(t5[:], t5_d.ap())
        kchunks = [x12T, cc4, x4aT, x4bT]
        for t in range(NT):
            hp = ps.tile([128, 512], F32, tag="h5")
            for kc in range(4):
                nc.tensor.matmul(hp[:], kchunks[kc][:, t * 128:(t + 1) * 128],
                                 w5[:, kc, :], start=(kc == 0), stop=False)
            nc.tensor.matmul(hp[:], ones_row[:, t * 128:(t + 1) * 128],
                             t5[:], start=False, stop=True)
            o_sb = sb.tile([128, 512], F32, tag="o_sb")
            nc.scalar.activation(o_sb[:], hp[:], AF.Relu)
            nc.sync.dma_start(out_d.ap()[t * 128:(t + 1) * 128, :], o_sb[:])

    return nc


_CACHED = {}


def _get_nc():
    if "nc" not in _CACHED:
        nc = bacc.Bacc("TRN2", target_bir_lowering=False, debug=False)
        build(nc)
        nc.compile()
        _CACHED["nc"] = nc
    return _CACHED["nc"]


def _in_maps(inputs):
    x = np.asarray(inputs["x"], dtype=np.float32)  # [8, 2048, 3]
    B = x.shape[0]
    common = {}
    for i, (C, D) in enumerate(BLOCKS):
        j = i + 1
        W = np.asarray(inputs[f"W{j}"], dtype=np.float32)
        g = np.asarray(inputs[f"g{j}"], dtype=np.float32)
        b = np.asarray(inputs[f"b{j}"], dtype=np.float32)
        m = np.asarray(inputs[f"m{j}"], dtype=np.float32)
        v = np.asarray(inputs[f"v{j}"], dtype=np.float32)
        s = (g / np.sqrt(v + EPS)).astype(np.float32)
        t = (b - m * s).astype(np.float32)
        assert (s > 0).all()
        common[f"Wc{j}"] = np.ascontiguousarray(W[:C])
        common[f"Wn{j}"] = np.ascontiguousarray(W[C:])
        common[f"s{j}"] = s.reshape(D, 1)
        common[f"t{j}"] = t.reshape(D, 1)
    W5 = np.asarray(inputs["W5"], dtype=np.float32)
    g5 = np.asarray(inputs["g5"], dtype=np.float32)
    b5 = np.asarray(inputs["b5"], dtype=np.float32)
    m5 = np.asarray(inputs["m5"], dtype=np.float32)
    v5 = np.asarray(inputs["v5"], dtype=np.float32)
    s5 = (g5 / np.sqrt(v5 + EPS)).astype(np.float32)
    t5 = (b5 - m5 * s5).astype(np.float32)
    common["W5s"] = np.ascontiguousarray(W5 * s5[None, :])
    common["t5"] = t5.reshape(1, 512)
    maps = []
    for c in range(B):
        mp = dict(common)
        mp["xT"] = np.ascontiguousarray(x[c].T)
        maps.append(mp)
    return maps


def kernel(**inputs) -> np.ndarray:
    nc = _get_nc()
    maps = _in_maps(inputs)
    res = bass_utils.run_bass_kernel_spmd(nc, maps, core_ids=list(range(len(maps))))
    out = np.stack([r["out"] for r in res.results])  # [8, 2048, 512]
    return out.astype(np.float32)


if __name__ == "__main__":
    _get_nc()
    print("compiled ok")
